# revision 9
# baseline (speedup 1.0000x reference)
"""EAST-style loss (weighted BCE score + smoothed-L1 geometry) on 8 trn2 cores.

Strategy: pure data parallel over batch m=128 -> 16 per core. Each core streams
its shard through SBUF once (memory-bound, 18 MiB/core at ~360 GB/s ~= 52.4 us),
computing per-partition partial sums with fused accumulate ops, all landing in
a single [128, NS] stats tile written out by one DMA. Final combine on host.

Geometry is processed as [128, 768/512] column chunks (3 per row-block) with a
4-op derived-DOF scheme (one relu op is reconstructed on the host):
  DVE:  d = yt - yp          (scalar_tensor_tensor, accum -> sum(d))
        c = clamp(d,-1,1)    (tensor_scalar min/max, accum -> sum(c))
  ACT:  square(c)            (accum -> sum(min(|d|,1)^2))
  alt op, alternating per chunk to balance engines:
    "dve": r1 = relu(d-1) = (d add -1) max 0   (accum -> sum r1)
           H_k = 0.5*sq + (cl - d) + 2*r1
    "act": r2 = relu(-d-1)   (Relu, scale=-1, bias=-1-tile; accum -> sum r2)
           H_k = 0.5*sq + (d - cl) + 2*r2
using the identity  d - clamp(d) = relu(d-1) - relu(-d-1).
This keeps DVE and ACT both under ~90% of the DMA slot for every chunk, so
pipeline lag always drains and the post-DMA tail stays short. The very last
load is a [128,320] slice of Y_true_score whose chain is 3 small DVE ops.
"""

import sys

sys.path.insert(0, "/opt/trn_rl_repo")

import numpy as np

import concourse.bacc as bacc
import concourse.mybir as mybir
from concourse.bass_utils import run_bass_kernel_spmd
from concourse.tile import TileContext

N_CORES = 8
M, H, W = 128, 128, 128
GC = 8  # geometry channels
M_PER = M // N_CORES  # 16

P = 128
F = 2048
GEOM_ELEMS = M_PER * GC * H * W  # 2097152
N_GT = GEOM_ELEMS // (P * F)  # 8 geometry row-blocks per core
FS = 320  # trailing yt_s slice width (the very last load)
FB = F - FS

# geometry plan: 3 column chunks per row-block, alt engine alternates
WIDTHS = [768, 768, 512]
N_CH = N_GT * len(WIDTHS)  # 24

# stats columns: per chunk (d, cl, sq, alt); then ln1m, ytA,t1A,t2A, ytB,t1B,t2B
C_LN1M = 4 * N_CH  # 96
C_YT_A = C_LN1M + 1
C_T1_A = C_LN1M + 2
C_T2_A = C_LN1M + 3
C_YT_B = C_LN1M + 4
C_T1_B = C_LN1M + 5
C_T2_B = C_LN1M + 6
NS = 128  # padded to 512B/partition so the stats DMA avoids the <512B penalty

F32 = mybir.dt.float32

_CACHED_NC = None


def _alt_kinds():
    kinds = []
    cur = "act"
    for _ in range(N_CH):
        kinds.append(cur)
        cur = "dve" if cur == "act" else "act"
    return kinds


ALT_KINDS = _alt_kinds()


def _build_nc():
    nc = bacc.Bacc("TRN2", target_bir_lowering=False)
    f32 = F32
    yt_s = nc.dram_tensor("yt_s", [P, F], f32, kind="ExternalInput")
    yp_s = nc.dram_tensor("yp_s", [P, F], f32, kind="ExternalInput")
    yt_g = nc.dram_tensor("yt_g", [N_GT, P, F], f32, kind="ExternalInput")
    yp_g = nc.dram_tensor("yp_g", [N_GT, P, F], f32, kind="ExternalInput")
    stats_d = nc.dram_tensor("stats", [P, NS], f32, kind="ExternalOutput")

    AF = mybir.ActivationFunctionType
    OP = mybir.AluOpType

    with TileContext(nc) as tc:
        with (
            tc.tile_pool(name="stats", bufs=1) as spool,
            tc.tile_pool(name="score", bufs=1) as scpool,
            tc.tile_pool(name="work", bufs=3) as wpool,
            tc.tile_pool(name="chio", bufs=8) as chpool,
            tc.tile_pool(name="chwork", bufs=8) as cwpool,
        ):
            st = spool.tile([P, NS], f32)
            cm1 = spool.tile([P, 1], f32)  # bias constant -1.0 for Relu(-d-1)
            nc.vector.memset(cm1[:], -1.0)

            # ---------------- score part (bulk) ----------------
            yp = scpool.tile([P, F], f32)
            nc.sync.dma_start(out=yp[:], in_=yp_s[:])
            yt = scpool.tile([P, F], f32)
            nc.sync.dma_start(out=yt[:, 0:FB], in_=yt_s[:, 0:FB])
            lnp = scpool.tile([P, F], f32)
            scr = wpool.tile([P, F], f32, tag="scr")
            # ln(yp)
            nc.scalar.activation(lnp[:], yp[:], AF.Ln)
            # ln(1-yp) in-place over yp; accum -> sum(ln(1-yp))
            nc.scalar.activation(
                yp[:], yp[:], AF.Ln, scale=-1.0, bias=1.0,
                accum_out=st[:, C_LN1M : C_LN1M + 1],
            )
            # sum(yt) first on DVE: absorbs the yt-DMA wait so the STT ops
            # below (limited sync-wait slots in the S2S2D2_STT struct) only
            # need a single ACT wait each.
            scr3 = wpool.tile([P, F], f32, tag="scr")
            nc.vector.tensor_scalar(
                out=scr3[:, 0:FB], in0=yt[:, 0:FB], scalar1=1.0, scalar2=0.0,
                op0=OP.mult, op1=OP.add,
                accum_out=st[:, C_YT_A : C_YT_A + 1],
            )
            # sum(yt * ln(yp))  (TTR hangs HW; STT accum works)
            nc.vector.scalar_tensor_tensor(
                out=scr[:, 0:FB], in0=yt[:, 0:FB], scalar=1.0, in1=lnp[:, 0:FB],
                op0=OP.mult, op1=OP.mult,
                accum_out=st[:, C_T1_A : C_T1_A + 1],
            )
            scr2 = wpool.tile([P, F], f32, tag="scr")
            # sum(yt * ln(1-yp))
            nc.vector.scalar_tensor_tensor(
                out=scr2[:, 0:FB], in0=yt[:, 0:FB], scalar=1.0, in1=yp[:, 0:FB],
                op0=OP.mult, op1=OP.mult,
                accum_out=st[:, C_T2_A : C_T2_A + 1],
            )

            # ---------------- geometry ----------------
            k = 0
            for i in range(N_GT):
                off = 0
                for fc in WIDTHS:
                    cs = slice(off, off + fc)
                    off += fc
                    alt = ALT_KINDS[k]
                    c0 = 4 * k
                    k += 1
                    a = chpool.tile([P, fc], f32, tag="ca")
                    nc.sync.dma_start(out=a[:], in_=yt_g[i][:, cs])
                    b = chpool.tile([P, fc], f32, tag="cb")
                    nc.sync.dma_start(out=b[:], in_=yp_g[i][:, cs])
                    d = cwpool.tile([P, fc], f32, tag="cd")
                    # d = (a * 1) - b, accum -> sum(d)
                    nc.vector.scalar_tensor_tensor(
                        out=d[:], in0=a[:], scalar=1.0, in1=b[:],
                        op0=OP.mult, op1=OP.subtract,
                        accum_out=st[:, c0 : c0 + 1],
                    )
                    mv = cwpool.tile([P, fc], f32, tag="cm")
                    # clamp(d,-1,1), accum -> sum(clamp)
                    nc.vector.tensor_scalar(
                        out=mv[:], in0=d[:], scalar1=1.0, scalar2=-1.0,
                        op0=OP.min, op1=OP.max,
                        accum_out=st[:, c0 + 1 : c0 + 2],
                    )
                    # square(clamp) in-place, accum
                    nc.scalar.activation(
                        mv[:], mv[:], AF.Square,
                        accum_out=st[:, c0 + 2 : c0 + 3],
                    )
                    if alt == "dve":
                        # r1 = relu(d-1); write over b (dead after sub)
                        nc.vector.tensor_scalar(
                            out=b[:], in0=d[:], scalar1=-1.0, scalar2=0.0,
                            op0=OP.add, op1=OP.max,
                            accum_out=st[:, c0 + 3 : c0 + 4],
                        )
                    else:
                        # r2 = relu(-d-1); write over a (dead after sub)
                        nc.scalar.activation(
                            a[:], d[:], AF.Relu, scale=-1.0, bias=cm1[:],
                            accum_out=st[:, c0 + 3 : c0 + 4],
                        )

            # ------- trailing yt_s slice: the very last load -------
            nc.sync.dma_start(out=yt[:, FB:F], in_=yt_s[:, FB:F])
            nc.vector.tensor_scalar(
                out=scr3[:, FB:F], in0=yt[:, FB:F], scalar1=1.0, scalar2=0.0,
                op0=OP.mult, op1=OP.add,
                accum_out=st[:, C_YT_B : C_YT_B + 1],
            )
            nc.vector.scalar_tensor_tensor(
                out=scr[:, FB:F], in0=yt[:, FB:F], scalar=1.0, in1=lnp[:, FB:F],
                op0=OP.mult, op1=OP.mult,
                accum_out=st[:, C_T1_B : C_T1_B + 1],
            )
            nc.vector.scalar_tensor_tensor(
                out=scr2[:, FB:F], in0=yt[:, FB:F], scalar=1.0, in1=yp[:, FB:F],
                op0=OP.mult, op1=OP.mult,
                accum_out=st[:, C_T2_B : C_T2_B + 1],
            )

            # split stats write: bulk columns (chunks 0..N_CH-3) go out early,
            # hidden under the tail compute; only a tiny final write (last two
            # chunks + score columns) sits on the critical path.
            csp = 4 * (N_CH - 2)
            nc.sync.dma_start(out=stats_d[:, 0:csp], in_=st[:, 0:csp])
            nc.sync.dma_start(out=stats_d[:, csp:NS], in_=st[:, csp:NS])
    nc.finalize()
    return nc


def _get_nc():
    global _CACHED_NC
    if _CACHED_NC is None:
        _CACHED_NC = _build_nc()
    return _CACHED_NC


def _make_in_maps(Y_true_score, Y_pred_score, Y_true_geometry, Y_pred_geometry):
    yts = np.ascontiguousarray(np.asarray(Y_true_score, dtype=np.float32))
    yps = np.ascontiguousarray(np.asarray(Y_pred_score, dtype=np.float32))
    ytg = np.ascontiguousarray(np.asarray(Y_true_geometry, dtype=np.float32))
    ypg = np.ascontiguousarray(np.asarray(Y_pred_geometry, dtype=np.float32))
    in_maps = []
    for k in range(N_CORES):
        sl = slice(k * M_PER, (k + 1) * M_PER)
        in_maps.append(
            {
                "yt_s": yts[sl].reshape(P, F),
                "yp_s": yps[sl].reshape(P, F),
                "yt_g": ytg[sl].reshape(N_GT, P, F),
                "yp_g": ypg[sl].reshape(N_GT, P, F),
            }
        )
    return in_maps


def _combine(results):
    """results: list of per-core dicts with stats [P, NS]."""
    huber_sum = 0.0
    ln1m_sum = 0.0
    t1_sum = 0.0
    t2_sum = 0.0
    yt_sum = 0.0
    for r in results:
        s = np.asarray(r["stats"], dtype=np.float64)
        g = s[:, 0 : 4 * N_CH].reshape(P, N_CH, 4).sum(axis=0)  # [N_CH, 4]
        for kk in range(N_CH):
            d_s, cl_s, sq_s, alt_s = g[kk]
            if ALT_KINDS[kk] == "dve":
                huber_sum += 0.5 * sq_s + (cl_s - d_s) + 2.0 * alt_s
            else:
                huber_sum += 0.5 * sq_s + (d_s - cl_s) + 2.0 * alt_s
        ln1m_sum += s[:, C_LN1M].sum()
        yt_sum += s[:, C_YT_A].sum() + s[:, C_YT_B].sum()
        t1_sum += s[:, C_T1_A].sum() + s[:, C_T1_B].sum()
        t2_sum += s[:, C_T2_A].sum() + s[:, C_T2_B].sum()

    size = float(M * 1 * H * W)
    beta = 1.0 - yt_sum / size
    A = t1_sum  # sum(yt * ln yp)
    B = ln1m_sum - t2_sum  # sum((1-yt) * ln(1-yp))
    loss_score = (-beta * A - (1.0 - beta) * B) / M

    n_pix = M * H * W
    loss_geom = huber_sum / GC / n_pix  # LAMBDA_GEOMETRY = 1.0

    return np.array(loss_score + loss_geom, dtype=np.float32)


def kernel(Y_true_score, Y_pred_score, Y_true_geometry, Y_pred_geometry, **_kw):
    nc = _get_nc()
    in_maps = _make_in_maps(
        Y_true_score, Y_pred_score, Y_true_geometry, Y_pred_geometry
    )
    res = run_bass_kernel_spmd(nc, in_maps, core_ids=list(range(N_CORES)))
    return _combine(res.results)
